# revision 20
# baseline (speedup 1.0000x reference)
"""Trainium2 Bass kernel for nn_AttentionModule (B=8, C=256, L=2048, D=32).

Per-batch computation (data-parallel: one batch per NeuronCore, 8 cores):
    qT = Wq @ x + bq            # (D, L)
    kT = Wk @ x + bk            # (D, L)
    vT = x.T @ (g*Wv).T + g*bv  # (L, C)   -- gamma folded into v
    ST = kT.T @ qT              # (L_j, L_i), row-packed 4x (D=32)
    E  = exp(ST)                # no max-subtraction: max|S| ~ 46, exp fits fp32
    Z  = sum_j E[j, i]          # DVE accumulate + PE ones-reduce to a row
    U  = vT.T @ E               # (C, L_i)  (already gamma-scaled)
    y  = U / Z + x

Final design (evolved over 10 trace-analysis iterations from a 97us
baseline; measures ~74.5us per core, ~87us when the shared chip sits in
the P0 power-throttle state):
  - Steady state is PE-bound (96% busy): per j-round 4 row-packed score
    MMs + 8 U MMs at N=512 (the PSUM-bank cap), ~2.5us vs ACT's 2.2us
    exp pair. Emission is software-pipelined with scores 2 rounds ahead
    of U; per-quarter tail work (Z, 1/Z, broadcast, finalize, y DMA)
    trails 1-3 rounds and is fully hidden except for quarter 3.
  - q/k projection uses column-tiled weights [Wq x4] / [Wk x4] so both
    come out of PSUM already replicated across the four 32-partition
    strips the packed score MMs need -- no gather/replica DMAs. q-copyback
    on ACT (bias via activation), k-copyback one DVE op per 512-col chunk
    into an un-gathered [128, 2048] layout the score lhsT slices directly.
  - 1/Z = exp(-ln Z) on ACT: Z is reduced to a [1,512] psum row by two
    ones-lhsT matmuls, ln+negated-exp run on ACT (both functions live in
    the natural_log_exp_and_others table set; the build filters the
    act-table map so the placement pass picks that one set instead of
    thrashing loads every quarter), then one outer-product matmul
    broadcasts the reciprocal row to [128,512]. (DVE reciprocal is ~8
    cyc/elem/lane -- usable only on [128,4]-shaped data; ACT is cheaper.)
  - GPSIMD is never used: its SBUF port steals ~3.5x DVE throughput.
  - Z accumulate: one bf16 TT per [128,1024] e-tile into a combined A|B
    accumulator; v bias folded in as a PE outer-product (ones x bv).
  - PSUM budget (8 banks): score ring 2x[128,1024] (4), U accumulators
    2x[128,512] (2), zrow/vt-scratch (1), rb/vt-scratch/prewarm (1).
  - Head: exp table prefetched via dummy ln/exp; PE prewarmed with 12
    full-array junk MMs through the input-DMA window (HAM un-throttles
    only on full-array activity); xb loaded in column-halves so qk and
    the first two score rounds start before half 1 lands; fp32 x
    (residual only) rides behind the compute inputs; DMA triggers
    (~0.7us each) split across the Sync and Scalar queues.
  - Tail: quarter 3's endgame is pipelined in two column-halves
    (ln/exp -> broadcast -> finalize -> y DMA on the idle Scalar queue).
"""

import numpy as np

B, C, L, D = 8, 256, 2048, 32
NCORES = 8

_cache = {}


def _build_nc():
    from contextlib import ExitStack

    import concourse.bacc as bacc
    import concourse.tile as tile
    from concourse import mybir

    f32 = mybir.dt.float32
    bf16 = mybir.dt.bfloat16
    EXP = mybir.ActivationFunctionType.Exp
    LN = mybir.ActivationFunctionType.Ln
    IDENT = mybir.ActivationFunctionType.Identity

    # Steer the act-table placement pass: Exp and Ln must both resolve to
    # natural_log_exp_and_others (one table load), not thrash between the
    # exp-only and ln-only sets. Only coverage is filtered -- set order and
    # ids stay canonical.
    _orig_tables = bacc.get_activation_tables

    def _tables_pref_lnexp(arch):
        t = dict(_orig_tables(arch))
        strip = {mybir.ActivationFunctionType.Exp, mybir.ActivationFunctionType.Ln}
        return {
            name: (fns if name == "natural_log_exp_and_others" else fns - strip)
            for name, fns in t.items()
        }

    bacc.get_activation_tables = _tables_pref_lnexp
    try:
        nc = _build_nc_inner(bacc, tile, mybir)
    finally:
        bacc.get_activation_tables = _orig_tables
    return nc


def _build_nc_inner(bacc, tile, mybir):
    from contextlib import ExitStack

    f32 = mybir.dt.float32
    bf16 = mybir.dt.bfloat16
    EXP = mybir.ActivationFunctionType.Exp
    LN = mybir.ActivationFunctionType.Ln
    IDENT = mybir.ActivationFunctionType.Identity

    nc = bacc.Bacc("TRN2", target_bir_lowering=False, debug=False)

    x_d = nc.dram_tensor("x", [C, L], f32, kind="ExternalInput")
    xb_d = nc.dram_tensor("xb", [C, L], bf16, kind="ExternalInput")
    wqk4_d = nc.dram_tensor("wqk4", [C, 256], bf16, kind="ExternalInput")
    wvT_d = nc.dram_tensor("wvT", [C, C], bf16, kind="ExternalInput")
    bqk4_d = nc.dram_tensor("bqk4", [128, 2], f32, kind="ExternalInput")
    bvb4_d = nc.dram_tensor("bvb4", [1, 4 * C], bf16, kind="ExternalInput")
    y_d = nc.dram_tensor("y", [C, L], f32, kind="ExternalOutput")

    x_ap = x_d.ap()
    y_ap = y_d.ap()

    with tile.TileContext(nc) as tc, ExitStack() as ctx:
        singles = ctx.enter_context(tc.tile_pool(name="singles", bufs=1))
        big = ctx.enter_context(tc.tile_pool(name="big", bufs=1))
        ps = ctx.enter_context(tc.tile_pool(name="ps", bufs=2, space="PSUM"))
        up = ctx.enter_context(tc.tile_pool(name="up", bufs=1, space="PSUM"))
        epool = ctx.enter_context(tc.tile_pool(name="epool", bufs=8))
        ypool = ctx.enter_context(tc.tile_pool(name="ypool", bufs=4))
        uspool = ctx.enter_context(tc.tile_pool(name="uspool", bufs=2))
        rpool = ctx.enter_context(tc.tile_pool(name="rpool", bufs=2))

        # ---- on-chip constants (padded to 64B/partition for alignment) ----
        ones_sb = singles.tile([128, 32], bf16, tag="ones")
        nc.vector.memset(ones_sb[:], 1.0)
        onesr_sb = singles.tile([1, 128], bf16, tag="onesr")
        nc.vector.memset(onesr_sb[:], 1.0)
        dummy_sb = singles.tile([1, 16], f32, tag="dummy")
        nc.vector.memset(dummy_sb[:], 1.0)
        # prefetch the ln+exp table set (natural_log_exp_and_others) while
        # input DMAs run; 1/Z later computes as exp(-ln Z) on ACT.
        nc.scalar.activation(dummy_sb[0:1, 1:2], dummy_sb[0:1, 0:1], LN)
        nc.scalar.activation(dummy_sb[0:1, 2:3], dummy_sb[0:1, 0:1], EXP)

        # ---- input DMA triggers ----
        # Scalar queue: small qk weights (needed first)
        wqk4_sb = []
        for ct in range(2):
            tq = singles.tile([128, 256], bf16, tag=f"wqk4{ct}")
            nc.scalar.dma_start(out=tq[:], in_=wqk4_d.ap()[ct * 128:(ct + 1) * 128, :])
            wqk4_sb.append(tq)
        bqk4_sb = singles.tile([128, 32], f32, tag="bqk4")
        nc.scalar.dma_start(out=bqk4_sb[:, 0:2], in_=bqk4_d.ap()[:, :])

        # Sync queue: xb in 512-col chunks so qk-it0/scores(0) start as early
        # as possible; v weights interleave right behind chunk 0.
        xb_sb = [big.tile([128, L], bf16, tag=f"xb{ct}", name=f"xb{ct}") for ct in range(2)]
        wvT_sb = [singles.tile([128, C], bf16, tag=f"wv{ct}", name=f"wv{ct}") for ct in range(2)]
        bvb4_sb = singles.tile([1, 4 * C], bf16, tag="bvb4")

        def xb_trig(it):
            for ct in range(2):
                nc.sync.dma_start(
                    out=xb_sb[ct][:, it * 512:(it + 1) * 512],
                    in_=xb_d.ap()[ct * 128:(ct + 1) * 128, it * 512:(it + 1) * 512],
                )

        xb_trig(0)
        for ct in range(2):
            nc.sync.dma_start(out=wvT_sb[ct][:], in_=wvT_d.ap()[ct * 128:(ct + 1) * 128, :])
        xb_trig(1)
        nc.sync.dma_start(out=bvb4_sb[:], in_=bvb4_d.ap()[:, :])
        xb_trig(2)
        xb_trig(3)
        x_sb = [big.tile([128, L], f32, tag=f"x{ct}", name=f"x{ct}") for ct in range(2)]
        for ct in range(2):
            nc.sync.dma_start(out=x_sb[ct][:], in_=x_ap[ct * 128:(ct + 1) * 128, :])

        # ---- PE prewarm: junk matmuls so HAM un-throttles before qk.
        # Needs >3.4us of sustained FULL-ARRAY activity (contraction-1
        # streams do not register with HAM) at the cold 1.2GHz clock.
        wfull_sb = singles.tile([128, 128], bf16, tag="wfull")
        nc.vector.memset(wfull_sb[:], 1.0)
        wbig_sb = singles.tile([128, 512], bf16, tag="wbig")
        nc.vector.memset(wbig_sb[:], 1.0)
        rbwarm = ps.tile([128, 512], f32, tag="rb", bufs=1, name="rbwarm")
        for w in range(12):
            nc.tensor.matmul(
                rbwarm[:, :], lhsT=wfull_sb[:], rhs=wbig_sb[:],
                start=True, stop=True,
            )

        # ---- q/k projection with strip-replication baked into the weights --
        # psum chunk: cols 0-511 = q replicated to 4 strips, 512-1023 = k
        # where strip g holds kT j-block {4*it+g} (exactly the kT4 layout).
        qT4x = big.tile([128, L], bf16, tag="qT4x")
        # kT4big chunk it holds kT[d, it*512+j'] on every strip; the score
        # matmul for round J slices out its strip's 128-col block. 4x the
        # SBUF of a gathered layout, but the k-copyback is one DVE op.
        kT4big = big.tile([128, 2048], bf16, tag="kT4big")

        def emit_qk(it):
            p = ps.tile([128, 1024], f32, tag="ps")
            for ct in range(2):
                nc.tensor.matmul(
                    p[:, 0:512],
                    lhsT=wqk4_sb[ct][:, 0:128],
                    rhs=xb_sb[ct][:, it * 512:(it + 1) * 512],
                    start=(ct == 0),
                    stop=(ct == 1),
                )
            for ct in range(2):
                nc.tensor.matmul(
                    p[:, 512:1024],
                    lhsT=wqk4_sb[ct][:, 128:256],
                    rhs=xb_sb[ct][:, it * 512:(it + 1) * 512],
                    start=(ct == 0),
                    stop=(ct == 1),
                )
            nc.scalar.activation(
                qT4x[:, it * 512:(it + 1) * 512], p[:, 0:512], IDENT,
                bias=bqk4_sb[:, 0:1],
            )
            nc.vector.tensor_scalar_add(
                kT4big[:, it * 512:(it + 1) * 512],
                p[:, 512:1024],
                bqk4_sb[:, 1:2],
            )

        # vT[j, c] as [128, 16*256]: block jb holds vT[jb*128 + p, c].
        vT_sb = big.tile([128, 16 * C], bf16, tag="vT")

        def emit_vt_group(grp, eng):
            # vt psum borrows the zmisc/rb banks (idle until the first
            # quarter boundary) so the ps ring stays free for scores.
            for hb in range(2):
                p = ps.tile([128, 512], f32, tag="zmisc" if hb == 0 else "rb",
                            bufs=1, name=f"vtp{hb}")
                # bias first: zeroes the bank with ones x (g*bv tiled)
                nc.tensor.matmul(
                    p[:, :],
                    lhsT=onesr_sb[:], rhs=bvb4_sb[0:1, hb * 512:(hb + 1) * 512],
                    start=True, stop=False, skip_group_check=True,
                )
                for lbr_rel in range(2):
                    lb = 4 * grp + 2 * hb + lbr_rel
                    for ct in range(2):
                        nc.tensor.matmul(
                            p[:, lbr_rel * C:(lbr_rel + 1) * C],
                            lhsT=xb_sb[ct][:, lb * 128:(lb + 1) * 128],
                            rhs=wvT_sb[ct][:],
                            start=False,
                            stop=(lbr_rel == 1 and ct == 1),
                            skip_group_check=True,
                        )
                nc.vector.tensor_copy(
                    vT_sb[:, grp * 1024 + hb * 512:grp * 1024 + (hb + 1) * 512],
                    p[:, :],
                )

        # ---- attention pipeline ----
        state = {}

        def emit_scores(t):
            qd, J = divmod(t, 4)
            i0 = qd * 512
            e_tiles = []
            for pair in range(2):
                stp = ps.tile([128, 1024], f32, tag="ps")
                for h in range(2):
                    g = 2 * pair + h
                    nc.tensor.matmul(
                        stp[:, h * 512:(h + 1) * 512],
                        lhsT=kT4big[32 * g:32 * (g + 1), J * 512 + g * 128:J * 512 + (g + 1) * 128],
                        rhs=qT4x[32 * g:32 * (g + 1), i0:i0 + 512],
                        start=True,
                        stop=True,
                        tile_position=(32 * g, 0),
                    )
                e2 = epool.tile([128, 1024], bf16, tag="e", name="e2")
                nc.scalar.activation(e2[:], stp[:], EXP)
                e_tiles.append(e2)
            state[t] = e_tiles

        def emit_u_zacc(u):
            qd, J = divmod(u, 4)
            if J == 0:
                state[f"u{qd}"] = [
                    up.tile([128, 512], f32, tag=f"u{ct}", name=f"u{ct}", bufs=1)
                    for ct in range(2)
                ]
                state[f"z{qd}"] = rpool.tile([128, 1024], bf16, tag="zacc", name="zacc")
            u_t = state[f"u{qd}"]
            e_tiles = state.pop(u)
            # ct-major: a late boundary u-copy of ct1 hides behind the ct0
            # matmuls. Last round goes pair-major instead so its first four
            # matmuls overlap the final exp.
            order = (
                [(ct, g) for ct in range(2) for g in range(4)] if u < 15 else
                [(ct, g) for g in range(4) for ct in range(2)]
            )
            for ct, g in order:
                jb = 4 * J + g
                eh = e_tiles[g // 2][:, (g % 2) * 512:(g % 2 + 1) * 512]
                nc.tensor.matmul(
                    u_t[ct][:, :],
                    lhsT=vT_sb[:, jb * C + ct * 128:jb * C + ct * 128 + 128],
                    rhs=eh,
                    start=(jb == 0),
                    stop=(jb == 15),
                )
            zacc = state[f"z{qd}"]
            for pair in range(2):
                if J == 0 and pair == 0:
                    nc.vector.tensor_copy(zacc[:], e_tiles[0][:])
                else:
                    nc.vector.tensor_add(zacc[:], zacc[:], e_tiles[pair][:])

        def emit_ucopy(qd):
            us = []
            for ct in range(2):
                u = uspool.tile([128, 512], f32, tag=f"us{ct}", name=f"us{ct}")
                nc.vector.tensor_copy(u[:], state[f"u{qd}"][ct][:, :])
                us.append(u)
            state[f"us{qd}"] = us

        def emit_zrow(qd):
            # Z as a row: ones.T @ zacc halves, accumulated in one psum row
            zacc = state[f"z{qd}"]
            zrow = ps.tile([1, 512], f32, tag="zmisc", bufs=1, name="zrow")
            nc.tensor.matmul(
                zrow[0:1, :], lhsT=ones_sb[:, 0:1], rhs=zacc[:, 0:512],
                start=True, stop=False,
            )
            nc.tensor.matmul(
                zrow[0:1, :], lhsT=ones_sb[:, 0:1], rhs=zacc[:, 512:1024],
                start=False, stop=True,
            )
            state[f"zrow{qd}"] = zrow

        def emit_rinv(qd):
            # 1/Z = exp(-ln Z), both in the natural_log_exp table set (ACT)
            zrow = state[f"zrow{qd}"]
            lnz = rpool.tile([1, 512], f32, tag="lnz", name="lnz")
            nc.scalar.activation(lnz[:], zrow[0:1, :], LN)
            rrow = rpool.tile([1, 512], bf16, tag="rrow", name="rrow")
            with nc.allow_low_precision(reason="1/Z in bf16: 0.4% rel on the attn term, well under tolerance"):
                nc.scalar.activation(rrow[:], lnz[:], EXP, scale=-1.0)
            state[f"rr{qd}"] = rrow

        def emit_rb(qd):
            rb_ps = ps.tile([128, 512], f32, tag="rb", bufs=1, name="rb_ps")
            nc.tensor.matmul(
                rb_ps[:, :], lhsT=onesr_sb[:], rhs=state[f"rr{qd}"][0:1, :],
                start=True, stop=True,
            )
            state[f"rb{qd}"] = rb_ps

        def emit_finalize(qd):
            i0 = qd * 512
            src = state[f"us{qd}"]
            rb = state[f"rb{qd}"]
            for ct in range(2):
                yt = ypool.tile([128, 512], f32, tag="y", name="yt")
                nc.vector.tensor_mul(yt[:], src[ct][:, :], rb[:, 0:512])
                nc.vector.tensor_add(yt[:], yt[:], x_sb[ct][:, i0:i0 + 512])
                nc.sync.dma_start(
                    out=y_ap[ct * 128:(ct + 1) * 128, i0:i0 + 512], in_=yt[:]
                )

        def emit_tail_last():
            # Quarter 3 endgame, fully exposed after the last exp: pipeline
            # 1/Z, broadcast, finalize and the y DMA in two column-halves.
            i0 = 3 * 512
            zrow = state["zrow3"]
            rb_ps = ps.tile([128, 512], f32, tag="rb", bufs=1, name="rb_ps")
            for h in range(2):
                cs = h * 256
                lnz = rpool.tile([1, 256], f32, tag=f"lnz3{h}", name="lnz")
                nc.scalar.activation(lnz[:], zrow[0:1, cs:cs + 256], LN)
                rrow = rpool.tile([1, 256], bf16, tag=f"rrow3{h}", name="rrow")
                with nc.allow_low_precision(reason="1/Z in bf16, well under tolerance"):
                    nc.scalar.activation(rrow[:], lnz[:], EXP, scale=-1.0)
                nc.tensor.matmul(
                    rb_ps[:, cs:cs + 256], lhsT=onesr_sb[:], rhs=rrow[0:1, :],
                    start=True, stop=True,
                )
                rb_sb = rpool.tile([128, 256], f32, tag=f"rbsb3{h}", name="rb_sb")
                nc.vector.tensor_copy(rb_sb[:], rb_ps[:, cs:cs + 256])
                for ct in range(2):
                    yt = ypool.tile([128, 256], f32, tag="ylast", name="yt")
                    nc.vector.tensor_mul(
                        yt[:], state["u3"][ct][:, cs:cs + 256], rb_sb[:]
                    )
                    nc.vector.tensor_add(
                        yt[:], yt[:], x_sb[ct][:, i0 + cs:i0 + cs + 256]
                    )
                    nc.scalar.dma_start(
                        out=y_ap[ct * 128:(ct + 1) * 128, i0 + cs:i0 + cs + 256],
                        in_=yt[:],
                    )

        # Head: scores(0)/(1) need only chunk-0 projections, so the exp
        # stream starts as soon as xb half 0 + qk land. vT groups follow
        # (copybacks on DVE so ACT stays pure-exp). Scores stay 2 rounds
        # ahead of U (lag-2) so only two U rounds trail the last exp.
        emit_qk(0)
        emit_scores(0)
        emit_vt_group(0, "dve")
        emit_qk(1)
        emit_scores(1)
        emit_vt_group(1, "dve")
        emit_qk(2)
        emit_qk(3)
        for u in range(16):
            qd, J = divmod(u, 4)
            emit_u_zacc(u)
            if u + 2 <= 15:
                emit_scores(u + 2)
            if u == 0:
                emit_vt_group(2, "dve")
            elif u == 1:
                emit_vt_group(3, "dve")
            if J == 3:
                if qd < 3:
                    emit_ucopy(qd)          # DVE: free u psum before next quarter
                emit_zrow(qd)               # PE ones-reduce
                if qd < 3:
                    emit_rinv(qd)           # ACT: 1/Z = exp(-ln Z)
            elif J == 0 and qd > 0:
                emit_rb(qd - 1)
            elif J == 1 and qd > 0:
                emit_finalize(qd - 1)
            if u == 15:
                emit_tail_last()

    nc.compile()
    return nc


def get_nc():
    if "nc" not in _cache:
        _cache["nc"] = _build_nc()
    return _cache["nc"]


def make_in_maps(x, Wq, bq, Wk, bk, Wv, bv, gamma):
    import ml_dtypes

    bf = ml_dtypes.bfloat16
    x = np.asarray(x, dtype=np.float32)
    g = float(np.asarray(gamma, np.float32).reshape(-1)[0])
    gbv = (g * np.asarray(bv, np.float32)).reshape(1, C)
    wq = np.asarray(Wq, np.float32).T        # (C, D)
    wk = np.asarray(Wk, np.float32).T        # (C, D)
    shared = {
        "wqk4": np.ascontiguousarray(
            np.concatenate([np.tile(wq, (1, 4)), np.tile(wk, (1, 4))], axis=1)
        ).astype(bf),
        "wvT": np.ascontiguousarray(g * np.asarray(Wv, np.float32).T).astype(bf),
        "bqk4": np.ascontiguousarray(np.stack(
            [np.tile(np.asarray(bq, np.float32).reshape(D), 4),
             np.tile(np.asarray(bk, np.float32).reshape(D), 4)], axis=1)),
        "bvb4": np.ascontiguousarray(np.tile(gbv, (1, 4))).astype(bf),
    }
    return [
        dict(shared, x=np.ascontiguousarray(x[b]), xb=np.ascontiguousarray(x[b]).astype(bf))
        for b in range(B)
    ]


def kernel(x, Wq, bq, Wk, bk, Wv, bv, gamma):
    from concourse.bass_utils import run_bass_kernel_spmd

    nc = get_nc()
    in_maps = make_in_maps(x, Wq, bq, Wk, bk, Wv, bv, gamma)
    res = run_bass_kernel_spmd(nc, in_maps, list(range(NCORES)))
    return np.stack([res.results[b]["y"] for b in range(B)], axis=0)


# revision 21
# speedup vs baseline: 1.0042x; 1.0042x over previous
"""Trainium2 Bass kernel for nn_AttentionModule (B=8, C=256, L=2048, D=32).

Per-batch computation (data-parallel: one batch per NeuronCore, 8 cores):
    qT = Wq @ x + bq            # (D, L)
    kT = Wk @ x + bk            # (D, L)
    vT = x.T @ (g*Wv).T + g*bv  # (L, C)   -- gamma folded into v
    ST = kT.T @ qT              # (L_j, L_i), row-packed 4x (D=32)
    E  = exp(ST)                # no max-subtraction: max|S| ~ 46, exp fits fp32
    Z  = sum_j E[j, i]          # DVE accumulate + PE ones-reduce to a row
    U  = vT.T @ E               # (C, L_i)  (already gamma-scaled)
    y  = U / Z + x

Final design (evolved over 10 trace-analysis iterations from a 97us
baseline; measures ~74.5us per core, ~87us when the shared chip sits in
the P0 power-throttle state):
  - Steady state is PE-bound (96% busy): per j-round 4 row-packed score
    MMs + 8 U MMs at N=512 (the PSUM-bank cap), ~2.5us vs ACT's 2.2us
    exp pair. Emission is software-pipelined with scores 2 rounds ahead
    of U; per-quarter tail work (Z, 1/Z, broadcast, finalize, y DMA)
    trails 1-3 rounds and is fully hidden except for quarter 3.
  - q/k projection uses column-tiled weights [Wq x4] / [Wk x4] so both
    come out of PSUM already replicated across the four 32-partition
    strips the packed score MMs need -- no gather/replica DMAs. q-copyback
    on ACT (bias via activation), k-copyback one DVE op per 512-col chunk
    into an un-gathered [128, 2048] layout the score lhsT slices directly.
  - 1/Z = exp(-ln Z) on ACT: Z is reduced to a [1,512] psum row by two
    ones-lhsT matmuls, ln+negated-exp run on ACT (both functions live in
    the natural_log_exp_and_others table set; the build filters the
    act-table map so the placement pass picks that one set instead of
    thrashing loads every quarter), then one outer-product matmul
    broadcasts the reciprocal row to [128,512]. (DVE reciprocal is ~8
    cyc/elem/lane -- usable only on [128,4]-shaped data; ACT is cheaper.)
  - GPSIMD is never used: its SBUF port steals ~3.5x DVE throughput.
  - Z accumulate: one bf16 TT per [128,1024] e-tile into a combined A|B
    accumulator; v bias folded in as a PE outer-product (ones x bv).
  - PSUM budget (8 banks): score ring 2x[128,1024] (4), U accumulators
    2x[128,512] (2), zrow/vt-scratch (1), rb/vt-scratch/prewarm (1).
  - Head: exp table prefetched via dummy ln/exp; PE prewarmed with 12
    full-array junk MMs through the input-DMA window (HAM un-throttles
    only on full-array activity); xb loaded in column-halves so qk and
    the first two score rounds start before half 1 lands; fp32 x
    (residual only) rides behind the compute inputs; DMA triggers
    (~0.7us each) split across the Sync and Scalar queues.
  - Tail: quarter 3's endgame is pipelined in two column-halves
    (ln/exp -> broadcast -> finalize -> y DMA on the idle Scalar queue).
"""

import numpy as np

B, C, L, D = 8, 256, 2048, 32
NCORES = 8

_cache = {}


def _build_nc():
    from contextlib import ExitStack

    import concourse.bacc as bacc
    import concourse.tile as tile
    from concourse import mybir

    f32 = mybir.dt.float32
    bf16 = mybir.dt.bfloat16
    EXP = mybir.ActivationFunctionType.Exp
    LN = mybir.ActivationFunctionType.Ln
    IDENT = mybir.ActivationFunctionType.Identity

    # Steer the act-table placement pass: Exp and Ln must both resolve to
    # natural_log_exp_and_others (one table load), not thrash between the
    # exp-only and ln-only sets. Only coverage is filtered -- set order and
    # ids stay canonical.
    _orig_tables = bacc.get_activation_tables

    def _tables_pref_lnexp(arch):
        t = dict(_orig_tables(arch))
        strip = {mybir.ActivationFunctionType.Exp, mybir.ActivationFunctionType.Ln}
        return {
            name: (fns if name == "natural_log_exp_and_others" else fns - strip)
            for name, fns in t.items()
        }

    bacc.get_activation_tables = _tables_pref_lnexp
    try:
        nc = _build_nc_inner(bacc, tile, mybir)
    finally:
        bacc.get_activation_tables = _orig_tables
    return nc


def _build_nc_inner(bacc, tile, mybir):
    from contextlib import ExitStack

    f32 = mybir.dt.float32
    bf16 = mybir.dt.bfloat16
    EXP = mybir.ActivationFunctionType.Exp
    LN = mybir.ActivationFunctionType.Ln
    IDENT = mybir.ActivationFunctionType.Identity

    nc = bacc.Bacc("TRN2", target_bir_lowering=False, debug=False)

    x_d = nc.dram_tensor("x", [C, L], f32, kind="ExternalInput")
    xb_d = nc.dram_tensor("xb", [C, L], bf16, kind="ExternalInput")
    wqk4_d = nc.dram_tensor("wqk4", [C, 256], bf16, kind="ExternalInput")
    wvT_d = nc.dram_tensor("wvT", [C, C], bf16, kind="ExternalInput")
    bqk4_d = nc.dram_tensor("bqk4", [128, 2], f32, kind="ExternalInput")
    bvr4_d = nc.dram_tensor("bvr4", [128, 4 * C], bf16, kind="ExternalInput")
    y_d = nc.dram_tensor("y", [C, L], f32, kind="ExternalOutput")

    x_ap = x_d.ap()
    y_ap = y_d.ap()

    with tile.TileContext(nc) as tc, ExitStack() as ctx:
        singles = ctx.enter_context(tc.tile_pool(name="singles", bufs=1))
        big = ctx.enter_context(tc.tile_pool(name="big", bufs=1))
        ps = ctx.enter_context(tc.tile_pool(name="ps", bufs=2, space="PSUM"))
        up = ctx.enter_context(tc.tile_pool(name="up", bufs=1, space="PSUM"))
        epool = ctx.enter_context(tc.tile_pool(name="epool", bufs=8))
        ypool = ctx.enter_context(tc.tile_pool(name="ypool", bufs=4))
        uspool = ctx.enter_context(tc.tile_pool(name="uspool", bufs=2))
        rpool = ctx.enter_context(tc.tile_pool(name="rpool", bufs=2))

        # ---- on-chip constants (padded to 64B/partition for alignment) ----
        ones_sb = singles.tile([128, 32], bf16, tag="ones")
        nc.vector.memset(ones_sb[:], 1.0)
        onesr_sb = singles.tile([1, 128], bf16, tag="onesr")
        nc.vector.memset(onesr_sb[:], 1.0)
        dummy_sb = singles.tile([1, 16], f32, tag="dummy")
        nc.vector.memset(dummy_sb[:], 1.0)
        # prefetch the ln+exp table set (natural_log_exp_and_others) while
        # input DMAs run; 1/Z later computes as exp(-ln Z) on ACT.
        nc.scalar.activation(dummy_sb[0:1, 1:2], dummy_sb[0:1, 0:1], LN)
        nc.scalar.activation(dummy_sb[0:1, 2:3], dummy_sb[0:1, 0:1], EXP)

        # ---- input DMA triggers ----
        # Scalar queue: small qk weights (needed first)
        wqk4_sb = []
        for ct in range(2):
            tq = singles.tile([128, 256], bf16, tag=f"wqk4{ct}")
            nc.scalar.dma_start(out=tq[:], in_=wqk4_d.ap()[ct * 128:(ct + 1) * 128, :])
            wqk4_sb.append(tq)
        bqk4_sb = singles.tile([128, 32], f32, tag="bqk4")
        nc.scalar.dma_start(out=bqk4_sb[:, 0:2], in_=bqk4_d.ap()[:, :])

        # Sync queue: xb in 512-col chunks so qk-it0/scores(0) start as early
        # as possible; v weights interleave right behind chunk 0.
        xb_sb = [big.tile([128, L], bf16, tag=f"xb{ct}", name=f"xb{ct}") for ct in range(2)]
        wvT_sb = [singles.tile([128, C], bf16, tag=f"wv{ct}", name=f"wv{ct}") for ct in range(2)]
        bvr4_sb = singles.tile([128, 4 * C], bf16, tag="bvr4")

        def xb_trig(it):
            for ct in range(2):
                nc.sync.dma_start(
                    out=xb_sb[ct][:, it * 512:(it + 1) * 512],
                    in_=xb_d.ap()[ct * 128:(ct + 1) * 128, it * 512:(it + 1) * 512],
                )

        xb_trig(0)
        for ct in range(2):
            nc.sync.dma_start(out=wvT_sb[ct][:], in_=wvT_d.ap()[ct * 128:(ct + 1) * 128, :])
        xb_trig(1)
        nc.sync.dma_start(out=bvr4_sb[:], in_=bvr4_d.ap()[:, :])
        xb_trig(2)
        xb_trig(3)
        x_sb = [big.tile([128, L], f32, tag=f"x{ct}", name=f"x{ct}") for ct in range(2)]
        for ct in range(2):
            nc.sync.dma_start(out=x_sb[ct][:], in_=x_ap[ct * 128:(ct + 1) * 128, :])

        # ---- PE prewarm: junk matmuls so HAM un-throttles before qk.
        # Needs >3.4us of sustained FULL-ARRAY activity (contraction-1
        # streams do not register with HAM) at the cold 1.2GHz clock.
        wfull_sb = singles.tile([128, 128], bf16, tag="wfull")
        nc.vector.memset(wfull_sb[:], 1.0)
        wbig_sb = singles.tile([128, 512], bf16, tag="wbig")
        nc.vector.memset(wbig_sb[:], 1.0)
        rbwarm = ps.tile([128, 512], f32, tag="rb", bufs=1, name="rbwarm")
        for w in range(12):
            nc.tensor.matmul(
                rbwarm[:, :], lhsT=wfull_sb[:], rhs=wbig_sb[:],
                start=True, stop=True,
            )

        # ---- q/k projection with strip-replication baked into the weights --
        # psum chunk: cols 0-511 = q replicated to 4 strips, 512-1023 = k
        # where strip g holds kT j-block {4*it+g} (exactly the kT4 layout).
        qT4x = big.tile([128, L], bf16, tag="qT4x")
        # kT4big chunk it holds kT[d, it*512+j'] on every strip; the score
        # matmul for round J slices out its strip's 128-col block. 4x the
        # SBUF of a gathered layout, but the k-copyback is one DVE op.
        kT4big = big.tile([128, 2048], bf16, tag="kT4big")

        def emit_qk(it):
            p = ps.tile([128, 1024], f32, tag="ps")
            for ct in range(2):
                nc.tensor.matmul(
                    p[:, 0:512],
                    lhsT=wqk4_sb[ct][:, 0:128],
                    rhs=xb_sb[ct][:, it * 512:(it + 1) * 512],
                    start=(ct == 0),
                    stop=(ct == 1),
                )
            for ct in range(2):
                nc.tensor.matmul(
                    p[:, 512:1024],
                    lhsT=wqk4_sb[ct][:, 128:256],
                    rhs=xb_sb[ct][:, it * 512:(it + 1) * 512],
                    start=(ct == 0),
                    stop=(ct == 1),
                )
            nc.scalar.activation(
                qT4x[:, it * 512:(it + 1) * 512], p[:, 0:512], IDENT,
                bias=bqk4_sb[:, 0:1],
            )
            nc.vector.tensor_scalar_add(
                kT4big[:, it * 512:(it + 1) * 512],
                p[:, 512:1024],
                bqk4_sb[:, 1:2],
            )

        # vT[j, c] as [128, 16*256]: block jb holds vT[jb*128 + p, c].
        vT_sb = big.tile([128, 16 * C], bf16, tag="vT")

        def emit_vt_group(grp, eng):
            # vt psum borrows the zmisc/rb banks (idle until the first
            # quarter boundary) so the ps ring stays free for scores.
            for hb in range(2):
                p = ps.tile([128, 512], f32, tag="zmisc" if hb == 0 else "rb",
                            bufs=1, name=f"vtp{hb}")
                for lbr_rel in range(2):
                    lb = 4 * grp + 2 * hb + lbr_rel
                    for ct in range(2):
                        nc.tensor.matmul(
                            p[:, lbr_rel * C:(lbr_rel + 1) * C],
                            lhsT=xb_sb[ct][:, lb * 128:(lb + 1) * 128],
                            rhs=wvT_sb[ct][:],
                            start=(ct == 0),
                            stop=(lbr_rel == 1 and ct == 1),
                            skip_group_check=True,
                        )
                # bias folds into the copyback for free (psum src is 1x anyway)
                nc.vector.tensor_add(
                    vT_sb[:, grp * 1024 + hb * 512:grp * 1024 + (hb + 1) * 512],
                    p[:, :],
                    bvr4_sb[:, hb * 512:(hb + 1) * 512],
                )

        # ---- attention pipeline ----
        state = {}

        def emit_scores(t):
            qd, J = divmod(t, 4)
            i0 = qd * 512
            e_tiles = []
            for pair in range(2):
                stp = ps.tile([128, 1024], f32, tag="ps")
                for h in range(2):
                    g = 2 * pair + h
                    nc.tensor.matmul(
                        stp[:, h * 512:(h + 1) * 512],
                        lhsT=kT4big[32 * g:32 * (g + 1), J * 512 + g * 128:J * 512 + (g + 1) * 128],
                        rhs=qT4x[32 * g:32 * (g + 1), i0:i0 + 512],
                        start=True,
                        stop=True,
                        tile_position=(32 * g, 0),
                    )
                e2 = epool.tile([128, 1024], bf16, tag="e", name="e2")
                nc.scalar.activation(e2[:], stp[:], EXP)
                e_tiles.append(e2)
            state[t] = e_tiles

        def emit_u_zacc(u):
            qd, J = divmod(u, 4)
            if J == 0:
                state[f"u{qd}"] = [
                    up.tile([128, 512], f32, tag=f"u{ct}", name=f"u{ct}", bufs=1)
                    for ct in range(2)
                ]
                state[f"z{qd}"] = rpool.tile([128, 1024], bf16, tag="zacc", name="zacc")
            u_t = state[f"u{qd}"]
            e_tiles = state.pop(u)
            # ct-major: a late boundary u-copy of ct1 hides behind the ct0
            # matmuls. Last round goes pair-major instead so its first four
            # matmuls overlap the final exp.
            order = (
                [(ct, g) for ct in range(2) for g in range(4)] if u < 15 else
                [(ct, g) for g in range(4) for ct in range(2)]
            )
            for ct, g in order:
                jb = 4 * J + g
                eh = e_tiles[g // 2][:, (g % 2) * 512:(g % 2 + 1) * 512]
                nc.tensor.matmul(
                    u_t[ct][:, :],
                    lhsT=vT_sb[:, jb * C + ct * 128:jb * C + ct * 128 + 128],
                    rhs=eh,
                    start=(jb == 0),
                    stop=(jb == 15),
                )
            zacc = state[f"z{qd}"]
            for pair in range(2):
                if J == 0 and pair == 0:
                    nc.vector.tensor_copy(zacc[:], e_tiles[0][:])
                else:
                    nc.vector.tensor_add(zacc[:], zacc[:], e_tiles[pair][:])

        def emit_ucopy(qd):
            us = []
            for ct in range(2):
                u = uspool.tile([128, 512], f32, tag=f"us{ct}", name=f"us{ct}")
                nc.vector.tensor_copy(u[:], state[f"u{qd}"][ct][:, :])
                us.append(u)
            state[f"us{qd}"] = us

        def emit_zrow(qd):
            # Z as a row: ones.T @ zacc halves, accumulated in one psum row
            zacc = state[f"z{qd}"]
            zrow = ps.tile([1, 512], f32, tag="zmisc", bufs=1, name="zrow")
            nc.tensor.matmul(
                zrow[0:1, :], lhsT=ones_sb[:, 0:1], rhs=zacc[:, 0:512],
                start=True, stop=False,
            )
            nc.tensor.matmul(
                zrow[0:1, :], lhsT=ones_sb[:, 0:1], rhs=zacc[:, 512:1024],
                start=False, stop=True,
            )
            state[f"zrow{qd}"] = zrow

        def emit_rinv(qd):
            # 1/Z = exp(-ln Z), both in the natural_log_exp table set (ACT)
            zrow = state[f"zrow{qd}"]
            lnz = rpool.tile([1, 512], f32, tag="lnz", name="lnz")
            nc.scalar.activation(lnz[:], zrow[0:1, :], LN)
            rrow = rpool.tile([1, 512], bf16, tag="rrow", name="rrow")
            with nc.allow_low_precision(reason="1/Z in bf16: 0.4% rel on the attn term, well under tolerance"):
                nc.scalar.activation(rrow[:], lnz[:], EXP, scale=-1.0)
            state[f"rr{qd}"] = rrow

        def emit_rb(qd):
            rb_ps = ps.tile([128, 512], f32, tag="rb", bufs=1, name="rb_ps")
            nc.tensor.matmul(
                rb_ps[:, :], lhsT=onesr_sb[:], rhs=state[f"rr{qd}"][0:1, :],
                start=True, stop=True,
            )
            state[f"rb{qd}"] = rb_ps

        def emit_finalize(qd):
            i0 = qd * 512
            src = state[f"us{qd}"]
            rb = state[f"rb{qd}"]
            for ct in range(2):
                yt = ypool.tile([128, 512], f32, tag="y", name="yt")
                nc.vector.tensor_mul(yt[:], src[ct][:, :], rb[:, 0:512])
                nc.vector.tensor_add(yt[:], yt[:], x_sb[ct][:, i0:i0 + 512])
                nc.sync.dma_start(
                    out=y_ap[ct * 128:(ct + 1) * 128, i0:i0 + 512], in_=yt[:]
                )

        def emit_tail_last():
            # Quarter 3 endgame, fully exposed after the last exp: pipeline
            # 1/Z, broadcast, finalize and the y DMA in two column-halves.
            i0 = 3 * 512
            zrow = state["zrow3"]
            rb_ps = ps.tile([128, 512], f32, tag="rb", bufs=1, name="rb_ps")
            for h in range(2):
                cs = h * 256
                lnz = rpool.tile([1, 256], f32, tag=f"lnz3{h}", name="lnz")
                nc.scalar.activation(lnz[:], zrow[0:1, cs:cs + 256], LN)
                rrow = rpool.tile([1, 256], bf16, tag=f"rrow3{h}", name="rrow")
                with nc.allow_low_precision(reason="1/Z in bf16, well under tolerance"):
                    nc.scalar.activation(rrow[:], lnz[:], EXP, scale=-1.0)
                nc.tensor.matmul(
                    rb_ps[:, cs:cs + 256], lhsT=onesr_sb[:], rhs=rrow[0:1, :],
                    start=True, stop=True,
                )
                rb_sb = rpool.tile([128, 256], f32, tag=f"rbsb3{h}", name="rb_sb")
                nc.vector.tensor_copy(rb_sb[:], rb_ps[:, cs:cs + 256])
                for ct in range(2):
                    yt = ypool.tile([128, 256], f32, tag="ylast", name="yt")
                    nc.vector.tensor_mul(
                        yt[:], state["u3"][ct][:, cs:cs + 256], rb_sb[:]
                    )
                    nc.vector.tensor_add(
                        yt[:], yt[:], x_sb[ct][:, i0 + cs:i0 + cs + 256]
                    )
                    nc.scalar.dma_start(
                        out=y_ap[ct * 128:(ct + 1) * 128, i0 + cs:i0 + cs + 256],
                        in_=yt[:],
                    )

        # Head: scores(0)/(1) need only chunk-0 projections, so the exp
        # stream starts as soon as xb half 0 + qk land. vT groups follow
        # (copybacks on DVE so ACT stays pure-exp). Scores stay 2 rounds
        # ahead of U (lag-2) so only two U rounds trail the last exp.
        emit_qk(0)
        emit_scores(0)
        emit_vt_group(0, "dve")
        emit_qk(1)
        emit_scores(1)
        emit_vt_group(1, "dve")
        emit_qk(2)
        emit_qk(3)
        for u in range(16):
            qd, J = divmod(u, 4)
            emit_u_zacc(u)
            if u + 2 <= 15:
                emit_scores(u + 2)
            if u == 0:
                emit_vt_group(2, "dve")
            elif u == 1:
                emit_vt_group(3, "dve")
            if J == 3:
                if qd < 3:
                    emit_ucopy(qd)          # DVE: free u psum before next quarter
                emit_zrow(qd)               # PE ones-reduce
                if qd < 3:
                    emit_rinv(qd)           # ACT: 1/Z = exp(-ln Z)
            elif J == 0 and qd > 0:
                emit_rb(qd - 1)
            elif J == 1 and qd > 0:
                emit_finalize(qd - 1)
            if u == 15:
                emit_tail_last()

    nc.compile()
    return nc


def get_nc():
    if "nc" not in _cache:
        _cache["nc"] = _build_nc()
    return _cache["nc"]


def make_in_maps(x, Wq, bq, Wk, bk, Wv, bv, gamma):
    import ml_dtypes

    bf = ml_dtypes.bfloat16
    x = np.asarray(x, dtype=np.float32)
    g = float(np.asarray(gamma, np.float32).reshape(-1)[0])
    gbv = (g * np.asarray(bv, np.float32)).reshape(1, C)
    wq = np.asarray(Wq, np.float32).T        # (C, D)
    wk = np.asarray(Wk, np.float32).T        # (C, D)
    shared = {
        "wqk4": np.ascontiguousarray(
            np.concatenate([np.tile(wq, (1, 4)), np.tile(wk, (1, 4))], axis=1)
        ).astype(bf),
        "wvT": np.ascontiguousarray(g * np.asarray(Wv, np.float32).T).astype(bf),
        "bqk4": np.ascontiguousarray(np.stack(
            [np.tile(np.asarray(bq, np.float32).reshape(D), 4),
             np.tile(np.asarray(bk, np.float32).reshape(D), 4)], axis=1)),
        "bvr4": np.ascontiguousarray(
            np.broadcast_to(np.tile(gbv, (1, 4)), (128, 4 * C))).astype(bf),
    }
    return [
        dict(shared, x=np.ascontiguousarray(x[b]), xb=np.ascontiguousarray(x[b]).astype(bf))
        for b in range(B)
    ]


def kernel(x, Wq, bq, Wk, bk, Wv, bv, gamma):
    from concourse.bass_utils import run_bass_kernel_spmd

    nc = get_nc()
    in_maps = make_in_maps(x, Wq, bq, Wk, bk, Wv, bv, gamma)
    res = run_bass_kernel_spmd(nc, in_maps, list(range(NCORES)))
    return np.stack([res.results[b]["y"] for b in range(B)], axis=0)
